# revision 11
# baseline (speedup 1.0000x reference)
"""GAT layer (segment-softmax message passing) on 8 Trainium2 NeuronCores.

v2 design — per-chunk indirect edge gather + matmul aggregation:

  - Nodes padded to 100352 = 784 tiles of 128; core c owns 98 tiles
    (nodes [c*12544, (c+1)*12544)).  Edges live on the core owning their
    dst; grouped by dst tile, chopped into chunks of 128 edges, sorted by
    src within a tile (gather locality).
  - Phase A (every core): z = h @ W^T in fp32 (784 GEMM tiles), rounded
    once to bf16, written to z_packed [100352 x 64] DRAM; the core's own
    98 tiles are re-gathered into SBUF (z_own).
  - Phase B per dst tile, per 128-edge chunk:
      * one indirect DMA (int32 row ids, one row per partition) gathers
        the chunk's z_src rows
      * ind[j,p] = (d[j]==p) via is_equal vs an iota tile (bf16 exact,
        8 chunks per DVE instruction)
      * ind_N = PE-transpose of ind; z_dst = ind_N^T @ z_own[t] (exact
        row selection on the tensor engine — no per-edge dst gather)
      * e = rowsum(z_src * z_dst) via one fused scalar_tensor_tensor
        with accum_out
    then per tile: leaky-relu; e-40 clamped at 80 (softmax is per-dst
    shift-invariant; keeps exp() finite on self-loop edges — the staged
    baseline corrupted dst 76141 exactly because exp(89) overflows);
    Exp on ACT; vals[j] = bf16(ex[j] * z_src[j]) with ex in column 64;
    agg[p,:] = sum_j ind[j,p] * vals[j,:] accumulated in PSUM per chunk;
    out = elu(agg[:,0:64] / agg[:,64]).

  Edge scalars and the aggregation never touch DRAM. Softmax logits use
  bf16-rounded z (sigma_e ~ 0.02 -> ~1-2% on attention weights); the
  harness gate is 2e-2.
"""

import os
import sys

sys.path.insert(0, "/opt/trn_rl_repo")

import numpy as np
import ml_dtypes

import concourse.bacc as bacc
import concourse.bass as bass
import concourse.mybir as mybir
import concourse.tile as tile
from concourse.bass_utils import run_bass_kernel_spmd
from concourse.masks import make_identity

F32 = mybir.dt.float32
BF16 = mybir.dt.bfloat16
I32 = mybir.dt.int32
AF = mybir.ActivationFunctionType
ALU = mybir.AluOpType
BF = ml_dtypes.bfloat16

LAST_RESULTS = None  # test harness reads exec_time_ns from here

NC = 8
E_SHIFT = 40.0
E_CLAMP = 80.0
G = 8  # chunks batched per DVE instruction


def _plan_core(src_g, dst_l, n_tiles, npc_base):
    t_arr = dst_l // 128
    order = np.lexsort((src_g, t_arr))
    ts = t_arr[order]
    ss = src_g[order]
    dl = (dst_l % 128)[order]

    counts = np.zeros(n_tiles, np.int64)
    d_blocks, s_blocks = [], []
    bounds = np.searchsorted(ts, np.arange(n_tiles + 1))
    for t in range(n_tiles):
        a, b = bounds[t], bounds[t + 1]
        m = b - a
        nch = max(1, (m + 127) // 128)
        pad = nch * 128
        d = np.full(pad, -1.0, np.float32)
        so = np.zeros(pad, np.int64)
        d[:m] = dl[a:b]
        so[:m] = ss[a:b]
        d_blocks.append(d.reshape(nch, 128).T)
        s_blocks.append(so.reshape(nch, 128).T)
        counts[t] = nch
    d_cols = np.concatenate(d_blocks, axis=1).astype(BF)
    s_cols = np.concatenate(s_blocks, axis=1).astype(np.int32)
    return d_cols, s_cols, counts


def _build(h, W, src, dst):
    h = np.asarray(h, np.float32)
    W = np.asarray(W, np.float32)
    src = np.asarray(src).astype(np.int64)
    dst = np.asarray(dst).astype(np.int64)

    N, IN_DIM = h.shape
    OUT_DIM = W.shape[0]
    assert IN_DIM == 128 and OUT_DIM == 64

    NT_G = ((N + 127) // 128 + NC - 1) // NC * NC
    NP = NT_G * 128
    T_OWN = NT_G // NC
    NPC = T_OWN * 128

    core_of = np.minimum(dst // NPC, NC - 1)
    plans = []
    for c in range(NC):
        m = core_of == c
        plans.append(_plan_core(src[m], (dst[m] - c * NPC).astype(np.int64),
                                T_OWN, c * NPC))

    counts_max = np.zeros(T_OWN, np.int64)
    for c in range(NC):
        counts_max = np.maximum(counts_max, plans[c][2])
    max_nch = int(counts_max.max())
    tot_cols = int(counts_max.sum())
    full_cols = tot_cols + T_OWN  # trailing cols: own-tile row ids

    hT_pad = np.zeros((IN_DIM, NP), np.float32)
    hT_pad[:, :N] = h.T
    wT = np.ascontiguousarray(W.T)
    iota_rep = np.tile(np.arange(128, dtype=np.float32), (128, 1)).astype(BF)

    in_maps = []
    for c in range(NC):
        d_cols, s_cols, counts = plans[c]
        dp = np.full((128, full_cols), -1.0, np.float32).astype(BF)
        op = np.zeros((128, full_cols), np.int32)
        s_off = 0
        d_off = 0
        for t in range(T_OWN):
            n = int(counts[t])
            dp[:, d_off:d_off + n] = d_cols[:, s_off:s_off + n]
            op[:, d_off:d_off + n] = s_cols[:, s_off:s_off + n]
            s_off += n
            d_off += int(counts_max[t])
        own = (c * NPC + np.arange(T_OWN)[None, :] * 128
               + np.arange(128)[:, None]).astype(np.int32)
        op[:, tot_cols:full_cols] = own
        in_maps.append({
            "hT": hT_pad,
            "wT": wT,
            "d_tab": dp,
            "off_tab": op,
            "iota_rep": iota_rep,
        })

    # ---- device program --------------------------------------------------
    nc = bacc.Bacc(None, target_bir_lowering=False, debug=False)
    hT_d = nc.declare_dram_parameter("hT", [IN_DIM, NP], F32, isOutput=False)
    wT_d = nc.declare_dram_parameter("wT", [IN_DIM, OUT_DIM], F32, isOutput=False)
    d_d = nc.declare_dram_parameter("d_tab", [128, full_cols], BF16, isOutput=False)
    off_d = nc.declare_dram_parameter("off_tab", [128, full_cols], I32, isOutput=False)
    iota_d = nc.declare_dram_parameter("iota_rep", [128, 128], BF16, isOutput=False)
    out_d = nc.declare_dram_parameter("out", [NPC, OUT_DIM], F32, isOutput=True)

    z_packed = nc.dram_tensor("z_packed", [NP, 2 * OUT_DIM], BF16)

    QB = 8

    with tile.TileContext(nc) as tc:
        with tc.tile_pool(name="const", bufs=1) as cpool, \
             tc.tile_pool(name="hst", bufs=3) as hpool, \
             tc.tile_pool(name="zps", bufs=2, space="PSUM") as zpspool, \
             tc.tile_pool(name="zst", bufs=3) as zpool, \
             tc.tile_pool(name="sg", bufs=3) as sgpool, \
             tc.tile_pool(name="ind", bufs=2) as indpool, \
             tc.tile_pool(name="tp", bufs=2, space="PSUM") as tppool, \
             tc.tile_pool(name="zd", bufs=2, space="PSUM") as zdpool, \
             tc.tile_pool(name="agg", bufs=2, space="PSUM") as aggpool, \
             tc.tile_pool(name="et", bufs=2) as epool, \
             tc.tile_pool(name="fin", bufs=2) as fpool:

            wt = cpool.tile([IN_DIM, OUT_DIM], F32, tag="wt")
            nc.sync.dma_start(wt[:], wT_d[:])
            iota_t = cpool.tile([128, 128], BF16, tag="iota")
            nc.sync.dma_start(iota_t[:], iota_d[:])
            ident = cpool.tile([128, 128], BF16, tag="ident")
            make_identity(nc, ident[:])
            z_own = cpool.tile([128, T_OWN, 2 * OUT_DIM], BF16, tag="zown")

            # ---------------- phase A: z = h @ W^T, bf16 ----------------
            for i0 in range(0, NT_G, QB):
                qb = min(QB, NT_G - i0)
                hstage = hpool.tile([IN_DIM, QB * 128], F32, tag="hstage")
                nc.sync.dma_start(hstage[:, : qb * 128],
                                  hT_d[:, i0 * 128:(i0 + qb) * 128])
                for j in range(qb):
                    gi = i0 + j
                    ps = zpspool.tile([128, OUT_DIM], F32, tag="zps")
                    nc.tensor.matmul(ps[:], hstage[:, j * 128:(j + 1) * 128],
                                     wt[:], start=True, stop=True)
                    zs = zpool.tile([128, 2 * OUT_DIM], BF16, tag="zstage")
                    nc.scalar.activation(zs[:, 0:OUT_DIM], ps[:], AF.Copy)
                    nc.vector.scalar_tensor_tensor(
                        out=zs[:, OUT_DIM:2 * OUT_DIM], in0=zs[:, 0:OUT_DIM],
                        scalar=-1.0, in1=ps[:], op0=ALU.mult, op1=ALU.add)
                    nc.sync.dma_start(z_packed[gi * 128:(gi + 1) * 128, :], zs[:])

            # own z tiles into SBUF
            own_off = cpool.tile([128, T_OWN], I32, tag="ownoff")
            nc.sync.dma_start(own_off[:], off_d[:, tot_cols:full_cols])
            for t in range(T_OWN):
                nc.gpsimd.indirect_dma_start(
                    out=z_own[:, t, :], out_offset=None,
                    in_=z_packed[:],
                    in_offset=bass.IndirectOffsetOnAxis(
                        ap=own_off[:, t:t + 1], axis=0),
                )

            # ---------------- phase B: per dst tile ---------------------
            col0 = 0
            for t in range(T_OWN):
                ncht = int(counts_max[t])
                dloc = epool.tile([128, max_nch], BF16, tag="dloc")
                nc.sync.dma_start(dloc[:, :ncht], d_d[:, col0:col0 + ncht])
                offs = epool.tile([128, max_nch], I32, tag="offs")
                nc.sync.dma_start(offs[:, :ncht], off_d[:, col0:col0 + ncht])

                sg = sgpool.tile([128, max_nch, 2 * OUT_DIM], BF16, tag="sg")
                for c in range(ncht):
                    nc.gpsimd.indirect_dma_start(
                        out=sg[:, c, :], out_offset=None,
                        in_=z_packed[:],
                        in_offset=bass.IndirectOffsetOnAxis(
                            ap=offs[:, c:c + 1], axis=0),
                    )

                ind_all = indpool.tile([128, max_nch, 128], BF16, tag="indall")
                for g0 in range(0, ncht, G):
                    gn = min(G, ncht - g0)
                    nc.vector.tensor_tensor(
                        out=ind_all[:, g0:g0 + gn, :],
                        in0=iota_t[:, None, :].broadcast_to((128, gn, 128)),
                        in1=dloc[:, g0:g0 + gn, None].broadcast_to((128, gn, 128)),
                        op=ALU.is_equal)

                e_t = epool.tile([128, max_nch], F32, tag="et")
                prod = indpool.tile([128, OUT_DIM], F32, tag="prod")
                s32_all = indpool.tile([128, max_nch, OUT_DIM], F32, tag="s32")
                for c in range(ncht):
                    tp = tppool.tile([128, 128], BF16, tag="tp")
                    nc.tensor.transpose(out=tp[:], in_=ind_all[:, c, :],
                                        identity=ident[:])
                    ind_n = indpool.tile([128, 128], BF16, tag="indn")
                    nc.scalar.activation(ind_n[:], tp[:], AF.Copy)
                    zd = zdpool.tile([128, OUT_DIM], F32, tag="zd")
                    nc.tensor.matmul(zd[:], ind_n[:], z_own[:, t, 0:OUT_DIM],
                                     start=True, stop=False)
                    nc.tensor.matmul(zd[:], ind_n[:], z_own[:, t, OUT_DIM:2 * OUT_DIM],
                                     start=False, stop=True)
                    nc.vector.tensor_tensor(
                        out=s32_all[:, c, :], in0=sg[:, c, 0:OUT_DIM],
                        in1=sg[:, c, OUT_DIM:2 * OUT_DIM], op=ALU.add)
                    nc.vector.scalar_tensor_tensor(
                        out=prod[:], in0=s32_all[:, c, :], scalar=1.0,
                        in1=zd[:], op0=ALU.mult, op1=ALU.mult,
                        accum_out=e_t[:, c:c + 1])

                # leaky-relu + shift/clamp + exp
                mx = epool.tile([128, max_nch], F32, tag="mx")
                nc.vector.tensor_scalar_max(mx[:, :ncht], e_t[:, :ncht], 0.0)
                mn = epool.tile([128, max_nch], F32, tag="mn")
                nc.vector.tensor_scalar_min(mn[:, :ncht], e_t[:, :ncht], 0.0)
                ls = epool.tile([128, max_nch], F32, tag="ls")
                nc.vector.scalar_tensor_tensor(
                    out=ls[:, :ncht], in0=mn[:, :ncht], scalar=0.2,
                    in1=mx[:, :ncht], op0=ALU.mult, op1=ALU.add)
                lc = epool.tile([128, max_nch], F32, tag="lc")
                nc.vector.tensor_scalar(
                    out=lc[:, :ncht], in0=ls[:, :ncht],
                    scalar1=-E_SHIFT, scalar2=E_CLAMP,
                    op0=ALU.add, op1=ALU.min)
                ex = epool.tile([128, max_nch], F32, tag="ex")
                nc.scalar.activation(ex[:, :ncht], lc[:, :ncht], AF.Exp)

                vals = sgpool.tile([128, max_nch, 65], BF16, tag="vals")
                for g0 in range(0, ncht, G):
                    gn = min(G, ncht - g0)
                    nc.vector.tensor_tensor(
                        out=vals[:, g0:g0 + gn, 0:OUT_DIM],
                        in0=s32_all[:, g0:g0 + gn, :],
                        in1=ex[:, g0:g0 + gn, None].broadcast_to((128, gn, OUT_DIM)),
                        op=ALU.mult)
                nc.vector.tensor_copy(vals[:, :ncht, OUT_DIM], ex[:, :ncht])

                agg = aggpool.tile([128, 65], F32, tag="agg")
                for c in range(ncht):
                    nc.tensor.matmul(agg[:], ind_all[:, c, :], vals[:, c, :],
                                     start=(c == 0), stop=(c == ncht - 1))

                # normalize + elu
                d1 = fpool.tile([128, 1], F32, tag="d1")
                nc.vector.tensor_scalar_add(d1[:], agg[:, 64:65], 1e-30)
                r = fpool.tile([128, 1], F32, tag="r")
                nc.vector.reciprocal(r[:], d1[:])
                o64 = fpool.tile([128, OUT_DIM], F32, tag="o64")
                nc.vector.tensor_scalar_mul(o64[:], agg[:, 0:64], r[:])
                mn2 = fpool.tile([128, OUT_DIM], F32, tag="mn2")
                nc.vector.tensor_scalar_min(mn2[:], o64[:], 0.0)
                emn = fpool.tile([128, OUT_DIM], F32, tag="emn")
                nc.scalar.activation(emn[:], mn2[:], AF.Exp)
                mx2 = fpool.tile([128, OUT_DIM], F32, tag="mx2")
                nc.vector.tensor_scalar_max(mx2[:], o64[:], 0.0)
                res = fpool.tile([128, OUT_DIM], F32, tag="res")
                nc.vector.scalar_tensor_tensor(
                    out=res[:], in0=emn[:], scalar=-1.0,
                    in1=mx2[:], op0=ALU.add, op1=ALU.add)
                nc.sync.dma_start(out_d[t * 128:(t + 1) * 128, :], res[:])

                col0 += ncht

    nc.finalize()
    return nc, in_maps, dict(NPC=NPC, N=N)


def kernel(h, W, src, dst):
    global LAST_RESULTS
    nc, in_maps, meta = _build(h, W, src, dst)
    results = run_bass_kernel_spmd(
        nc, in_maps, core_ids=list(range(NC)),
        trace=bool(int(os.environ.get("GAT_TRACE", "0"))),
    )
    LAST_RESULTS = results
    NPC, N = meta["NPC"], meta["N"]
    parts = []
    for c in range(NC):
        hi = min(NPC, N - c * NPC)
        parts.append(results.results[c]["out"][:hi])
    return np.concatenate(parts, axis=0).astype(np.float32)
